# revision 1
# baseline (speedup 1.0000x reference)
"""Trainium2 Bass kernel for causal attention (B=4, S=4096, D_IN=1024, D_OUT=64).

Collective-free design: 2 cores per batch, query rows interleaved at 128-row
tile granularity (parity j = core % 2 takes global q-tiles j::2). Each core
computes K^T/V^T for the FULL batch itself (duplicated projections are far
cheaper than the AllGather they replace), Q^T only for its own q-tiles.
x^T is pre-transposed and tile-reordered on the host so the device does zero
transposition of x; all matmuls bf16 with f32 PSUM accumulation.

Storage is in core-relative "slots": own slot r holds global k-tile 2r+j,
peer slot r holds 2r+(1-j). For local q-tile t (global 2t+j) the causal set
is exactly own slots 0..t plus peer slots 0..t, with a per-core mask on the
final pair (tri on own diagonal; zeros/ones on the peer slot) — so the SPMD
program is core-independent and fully balanced.

Schedule: x^T chunk DMAs are issued first (behind a small identity load that
feeds PE warmup matmuls, keeping the PE p-state ramped while DMAs land).
Projection chunk 0 runs up front; projection chunk c+1 is then interleaved
into attention group c's unit loop (popping from a work deque between
score/AV units) so the exp-gated PE bubbles are filled with projection
matmuls. Attention per unit: scores (K^T.T @ Q^T) -> exp on ScalarE
(scale=1/8 folded) -> out^T[65] += [V|1].T @ P^T with a 2-unit score
lookahead; rowsum rides in the AV matmul via an embedded ones column.
Host divides by the rowsum row, transposes, and reassembles.
"""

import numpy as np
import ml_dtypes

import concourse.bass as bass
import concourse.tile as tile
from concourse import bacc, mybir
from concourse.bass_utils import run_bass_kernel_spmd

B, S, D_IN, D_OUT = 4, 4096, 1024, 64
N_CORES = 8
NCHUNK = 4             # s-chunks: chunk c covers global s-tiles 8c..8c+7
SCALE = 1.0 / 8.0      # 1/sqrt(D_OUT)
BF16 = mybir.dt.bfloat16
F32 = mybir.dt.float32
UNIT_OFFSET = [0, 4, 12, 24]   # global attention-unit index at each group start


def kcol(r, peer):
    """Column of k-slot r (own/peer) in the [64, 4096] K^T / V^T tiles."""
    return 1024 * (r // 4) + 512 * peer + 128 * (r % 4)


def vslot(r, peer):
    """v_view slot index of k-slot r."""
    return 8 * (r // 4) + 4 * peer + (r % 4)


def build_kernel(nc, tc, xs, wqk, wkv, wv, msk, ident, vinit, outT):
    from collections import deque
    from contextlib import ExitStack
    ctx = ExitStack()
    const_pool = ctx.enter_context(tc.tile_pool(name="const", bufs=1))
    ident_sb = const_pool.tile([128, 128], BF16, name="ident_sb")
    msk_sb = const_pool.tile([128, 256], BF16, name="msk_sb")
    wqk_sb = const_pool.tile([128, 1024], BF16, name="wqk_sb")  # cols 128k: [wq_k | wk_k]
    wkv_sb = const_pool.tile([128, 1024], BF16, name="wkv_sb")  # cols 128k: [wk_k | wv_k]
    wv_sb = const_pool.tile([128, 512], BF16, name="wv_sb")     # cols 64k: wv_k

    big_pool = ctx.enter_context(tc.tile_pool(name="big", bufs=1))
    xT = big_pool.tile([128, 4 * 8192], BF16, name="xT")   # chunk c at 8192c; k at +1024k; [own512|peer512]
    qT = big_pool.tile([64, 2048], BF16, name="qT")        # local tile t at 128t
    kT = big_pool.tile([64, 4096], BF16, name="kT")        # slot layout via kcol()
    vT = big_pool.tile([64, 4096], BF16, name="vT")
    v_all = big_pool.tile([128, 32 * 66], BF16, name="v_all")  # V natural slots + ones col
    outT_sb = big_pool.tile([65, 2048], F32, name="outT_sb")

    pt_pool = ctx.enter_context(tc.tile_pool(name="pt", bufs=5))

    # PSUM: scores 1 bank x bufs=4 = 4; acc 1 x 2 = 2; proj/vnat 1 x 2 = 2.
    sc_pool = ctx.enter_context(tc.tile_pool(name="scp", bufs=4, space="PSUM"))
    acc_pool = ctx.enter_context(tc.tile_pool(name="accp", bufs=2, space="PSUM"))
    pj_pool = ctx.enter_context(tc.tile_pool(name="pjp", bufs=2, space="PSUM"))

    # Small constants first (weights gate chunk-0 projections), then the x
    # chunks in 2048-col pieces alternating across BOTH hardware DMA queues
    # (SP + Activation): double bandwidth, and projection k-slices can start
    # as soon as their piece lands.
    nc.sync.dma_start(ident_sb[:], ident[:])
    nc.sync.dma_start(wqk_sb[:], wqk[:])
    nc.scalar.dma_start(wkv_sb[:], wkv[:])
    nc.scalar.dma_start(wv_sb[:], wv[:])
    nc.scalar.dma_start(msk_sb[:], msk[:])
    for p in range(16):
        eng = nc.sync if p % 2 == 0 else nc.scalar
        eng.dma_start(xT[:, 2048 * p: 2048 * (p + 1)], xs[:, 2048 * p: 2048 * (p + 1)])
        if p == 3:
            nc.sync.dma_start(v_all[:], vinit[:])

    v_view = v_all[:].rearrange("p (u e) -> p u e", e=66)
    ident64 = ident_sb[0:64, 0:64]

    # PE warmup on a memset tile (no DMA dependency): keeps the PE p-state
    # ramped while the x DMAs land.
    warm_sb = const_pool.tile([128, 128], BF16, name="warm_sb")
    nc.vector.memset(warm_sb[:], 0.0)
    warm_ps = pj_pool.tile([128, 512], F32, tag="pj")
    for i in range(24):
        nc.tensor.matmul(warm_ps[:, 0:128], warm_sb[:], warm_sb[:],
                         start=(i == 0), stop=(i == 23))

    def proj_steps(c):
        """Projection of chunk c (global s-tiles 8c..8c+7) as a list of
        emission thunks, so attention can interleave them into PE bubbles."""
        base = 8192 * c
        ccols = slice(1024 * c, 1024 * c + 512)
        pcols = slice(1024 * c + 512, 1024 * c + 1024)
        steps = []
        holder = {}

        def qk_mm(k):
            def f():
                if k == 0:
                    holder["qk"] = pj_pool.tile([128, 512], F32, name=f"qk_ps{c}", tag="pj")
                xo = xT[:, base + 1024 * k: base + 1024 * k + 512]
                nc.tensor.matmul(holder["qk"][:], wqk_sb[:, 128 * k: 128 * (k + 1)], xo,
                                 start=(k == 0), stop=(k == 7))
                if k == 7:
                    qk = holder.pop("qk")
                    nc.vector.tensor_copy(qT[:, 512 * c: 512 * (c + 1)], qk[0:64, :])
                    nc.vector.tensor_copy(kT[:, ccols], qk[64:128, :])
            return f

        def kv_mm(k):
            def f():
                if k == 0:
                    holder["kv"] = pj_pool.tile([128, 512], F32, name=f"kv_ps{c}", tag="pj")
                xp = xT[:, base + 1024 * k + 512: base + 1024 * k + 1024]
                nc.tensor.matmul(holder["kv"][:], wkv_sb[:, 128 * k: 128 * (k + 1)], xp,
                                 start=(k == 0), stop=(k == 7))
                if k == 7:
                    kv = holder.pop("kv")
                    nc.vector.tensor_copy(kT[:, pcols], kv[0:64, :])
                    nc.vector.tensor_copy(vT[:, pcols], kv[64:128, :])
            return f

        def v_mm(k):
            def f():
                if k == 0:
                    holder["v"] = pj_pool.tile([64, 512], F32, name=f"v_ps{c}", tag="pj")
                xo = xT[:, base + 1024 * k: base + 1024 * k + 512]
                nc.tensor.matmul(holder["v"][:], wv_sb[:, 64 * k: 64 * k + 64], xo,
                                 start=(k == 0), stop=(k == 7))
                if k == 7:
                    nc.vector.tensor_copy(vT[:, ccols], holder.pop("v")[:, :])
            return f

        def vn_mm(m):
            def f():
                if m == 0:
                    holder["vn"] = pj_pool.tile([128, 512], F32, name=f"vn_ps{c}", tag="pj")
                nc.tensor.matmul(holder["vn"][:, 64 * m: 64 * m + 64],
                                 vT[:, 1024 * c + 128 * m: 1024 * c + 128 * (m + 1)],
                                 ident64, start=True, stop=True)
                nc.tensor.matmul(holder["vn"][:, 256 + 64 * m: 256 + 64 * m + 64],
                                 vT[:, 1024 * c + 512 + 128 * m: 1024 * c + 512 + 128 * (m + 1)],
                                 ident64, start=True, stop=True)
                if m == 3:
                    vn = holder.pop("vn")
                    nc.vector.tensor_copy(
                        v_view[:, 8 * c: 8 * c + 8, 0:64],
                        vn[:].rearrange("p (u e) -> p u e", e=64),
                    )
            return f

        # (deadline, fn): qk feeds group c's own q-tiles, needed by its
        # prologue (2-unit lookahead); the rest only by group c's tail.
        dl_qk = UNIT_OFFSET[c] - 2
        dl_rest = UNIT_OFFSET[c] + 4 * c - 2
        for k in range(8):
            steps.append((dl_qk, qk_mm(k)))
        for k in range(8):
            steps.append((dl_rest, kv_mm(k)))
        for k in range(8):
            steps.append((dl_rest, v_mm(k)))
        for m in range(4):
            steps.append((dl_rest, vn_mm(m)))
        return steps

    work = deque()

    def pace(U):
        """Pop overdue steps, then pace the front deadline-run so it drains
        evenly by its deadline."""
        while work and work[0][0] <= U:
            work.popleft()[1]()
        if not work:
            return
        dl = work[0][0]
        n = 0
        for d, _ in work:
            if d != dl:
                break
            n += 1
        k = -(-n // (dl - U))
        for _ in range(k):
            work.popleft()[1]()

    def attention_group(g):
        """Local q-tiles 4g..4g+3 (q-cols 512g..512g+512), software-pipelined
        with 2-unit score lookahead; proj steps interleaved from `work`."""
        acc = acc_pool.tile([65, 512], F32, tag="acc")
        qcols = slice(512 * g, 512 * (g + 1))
        n_units = 4 * g + 4
        state = {}

        def scores(u):
            if u < 4 * g:  # bulk pair: own slot u + peer slot u, all 512 q-cols
                so = sc_pool.tile([128, 512], F32, name=f"so{g}_{u}", tag="sc")
                sp_ = sc_pool.tile([128, 512], F32, name=f"spp{g}_{u}", tag="sc")
                nc.tensor.matmul(so[:], kT[:, kcol(u, 0): kcol(u, 0) + 128],
                                 qT[:, qcols], start=True, stop=True)
                nc.tensor.matmul(sp_[:], kT[:, kcol(u, 1): kcol(u, 1) + 128],
                                 qT[:, qcols], start=True, stop=True)
                po = pt_pool.tile([128, 512], BF16, name=f"po{g}_{u}", tag="pt")
                pp = pt_pool.tile([128, 512], BF16, name=f"pp{g}_{u}", tag="pt")
                nc.scalar.activation(po[:], so[:], mybir.ActivationFunctionType.Exp, scale=SCALE)
                nc.scalar.activation(pp[:], sp_[:], mybir.ActivationFunctionType.Exp, scale=SCALE)
                state[u] = (po, pp)
            else:          # tail tile t = u: pairs r = 4g..t, one 128-col q-tile
                t = u
                a = t - 4 * g
                width = 256 * (a + 1)
                qc = slice(128 * t, 128 * (t + 1))
                half = min(width, 512)
                s0 = sc_pool.tile([128, 512], F32, name=f"s0_{g}_{u}", tag="sc")
                s1 = sc_pool.tile([128, 512], F32, name=f"s1_{g}_{u}", tag="sc") if width > 512 else None
                for rr in range(4 * g, t + 1):
                    off = 256 * (rr - 4 * g)
                    stb, o = (s0, off) if off < 512 else (s1, off - 512)
                    nc.tensor.matmul(stb[:, o: o + 128], kT[:, kcol(rr, 0): kcol(rr, 0) + 128],
                                     qT[:, qc], start=True, stop=True)
                    nc.tensor.matmul(stb[:, o + 128: o + 256], kT[:, kcol(rr, 1): kcol(rr, 1) + 128],
                                     qT[:, qc], start=True, stop=True)
                p0 = pt_pool.tile([128, 512], BF16, name=f"p0_{g}_{u}", tag="pt")
                p1 = pt_pool.tile([128, 512], BF16, name=f"p1_{g}_{u}", tag="pt") if s1 is not None else None
                nc.scalar.activation(p0[:, 0:half], s0[:, 0:half],
                                     mybir.ActivationFunctionType.Exp, scale=SCALE)
                if s1 is not None:
                    nc.scalar.activation(p1[:, 0:width - 512], s1[:, 0:width - 512],
                                         mybir.ActivationFunctionType.Exp, scale=SCALE)
                # mask the final pair (cols width-256..width of the logical row)
                mt, mo = (p0, width - 256) if width <= 512 else (p1, width - 768)
                nc.vector.tensor_mul(mt[:, mo: mo + 256], mt[:, mo: mo + 256], msk_sb[:])
                state[u] = (p0, p1)

        def av(u):
            if u < 4 * g:
                po, pp = state.pop(u)
                nc.tensor.matmul(acc[:], v_view[:, vslot(u, 0), 0:65], po[:],
                                 start=(u == 0), stop=False)
                nc.tensor.matmul(acc[:], v_view[:, vslot(u, 1), 0:65], pp[:],
                                 start=False, stop=False)
            else:
                t = u
                a = t - 4 * g
                p0, p1 = state.pop(u)
                ac = acc[0:65, 128 * a: 128 * (a + 1)]
                for rr in range(4 * g, t + 1):
                    off = 256 * (rr - 4 * g)
                    pt_, o = (p0, off) if off < 512 else (p1, off - 512)
                    st = (g == 0 and rr == 0)
                    sp = (rr == t)
                    nc.tensor.matmul(ac, v_view[:, vslot(rr, 0), 0:65],
                                     pt_[:, o: o + 128], start=st, stop=False)
                    nc.tensor.matmul(ac, v_view[:, vslot(rr, 1), 0:65],
                                     pt_[:, o + 128: o + 256], start=False, stop=sp)
                # this tile's output column is final: drain it now
                oc = slice(128 * t, 128 * (t + 1))
                nc.vector.tensor_copy(outT_sb[:, oc], acc[0:65, 128 * a: 128 * (a + 1)])
                nc.sync.dma_start(outT[:, oc], outT_sb[:, oc])

        scores(0)
        if n_units > 1:
            scores(1)
        for u in range(n_units):
            if u + 2 < n_units:
                scores(u + 2)
            pace(UNIT_OFFSET[g] + u)
            av(u)

    for _, f in proj_steps(0):
        f()
    for g in range(NCHUNK):
        if g + 1 < NCHUNK:
            work.extend(proj_steps(g + 1))
        attention_group(g)
    assert not work

    ctx.close()


def build_nc():
    nc = bacc.Bacc("TRN2", target_bir_lowering=False, debug=False,
                   num_devices=N_CORES)
    xs = nc.dram_tensor("xs", [128, 4 * 8192], BF16, kind="ExternalInput").ap()
    wqk = nc.dram_tensor("wqk", [128, 1024], BF16, kind="ExternalInput").ap()
    wkv = nc.dram_tensor("wkv", [128, 1024], BF16, kind="ExternalInput").ap()
    wv = nc.dram_tensor("wv", [128, 512], BF16, kind="ExternalInput").ap()
    msk = nc.dram_tensor("msk", [128, 256], BF16, kind="ExternalInput").ap()
    ident = nc.dram_tensor("ident", [128, 128], BF16, kind="ExternalInput").ap()
    vinit = nc.dram_tensor("vinit", [128, 32 * 66], BF16, kind="ExternalInput").ap()
    outT = nc.dram_tensor("outT", [65, 2048], F32, kind="ExternalOutput").ap()
    with tile.TileContext(nc) as tc:
        build_kernel(nc, tc, xs, wqk, wkv, wv, msk, ident, vinit, outT)
    nc.compile()
    return nc


_NC_CACHE = None


def get_nc():
    global _NC_CACHE
    if _NC_CACHE is None:
        _NC_CACHE = build_nc()
    return _NC_CACHE


def make_in_maps(x, Wq, Wk, Wv):
    bf = ml_dtypes.bfloat16
    wqkT = np.concatenate([Wq.T.reshape(8, 128, 64), Wk.T.reshape(8, 128, 64)], axis=2)
    wqk = np.ascontiguousarray(wqkT.transpose(1, 0, 2).reshape(128, 1024).astype(bf))
    wkvT = np.concatenate([Wk.T.reshape(8, 128, 64), Wv.T.reshape(8, 128, 64)], axis=2)
    wkv = np.ascontiguousarray(wkvT.transpose(1, 0, 2).reshape(128, 1024).astype(bf))
    wvT = Wv.T.reshape(8, 128, 64)
    wv = np.ascontiguousarray(wvT.transpose(1, 0, 2).reshape(128, 512).astype(bf))
    ident = np.eye(128, dtype=bf)
    vinit = np.ones((128, 32 * 66), dtype=bf)
    tri = np.triu(np.ones((128, 128), dtype=np.float32))  # [k, q] = 1 if k <= q
    tri = tri.astype(bf)
    ones = np.ones((128, 128), dtype=bf)
    zeros = np.zeros((128, 128), dtype=bf)
    msk_even = np.ascontiguousarray(np.concatenate([tri, zeros], axis=1))
    msk_odd = np.ascontiguousarray(np.concatenate([tri, ones], axis=1))
    in_maps = []
    for core in range(N_CORES):
        b, j = core // 2, core % 2
        # [s_tile 32, s_in 128, k 8, i_in 128]
        y = x[b].reshape(32, 128, 8, 128)
        t_idx = np.array([[8 * c + j + 2 * m for m in range(4)] +
                          [8 * c + (1 - j) + 2 * m for m in range(4)]
                          for c in range(4)])                      # [4, 8]
        z = y[t_idx]                       # [c, hm, s_in, k, i_in]
        z = z.transpose(4, 0, 3, 1, 2)     # [i_in, c, k, hm, s_in]
        xsh = np.ascontiguousarray(z.reshape(128, 4 * 8192).astype(bf))
        in_maps.append({
            "xs": xsh,
            "wqk": wqk,
            "wkv": wkv,
            "wv": wv,
            "msk": msk_even if j == 0 else msk_odd,
            "ident": ident,
            "vinit": vinit,
        })
    return in_maps


def assemble_output(results):
    out = np.empty((B, S, D_OUT), dtype=np.float32)
    for c in range(N_CORES):
        b, j = c // 2, c % 2
        oT = results[c]["outT"].astype(np.float32)  # [65, 2048]
        o = (oT[:64] / oT[64:65]).T                 # [2048, 64]
        out[b].reshape(32, 128, D_OUT)[j::2] = o.reshape(16, 128, D_OUT)
    return out


def kernel(x, Wq, Wk, Wv):
    nc = get_nc()
    in_maps = make_in_maps(np.asarray(x), np.asarray(Wq), np.asarray(Wk), np.asarray(Wv))
    res = run_bass_kernel_spmd(nc, in_maps, core_ids=list(range(N_CORES)))
    return assemble_output(res.results)

